# revision 19
# baseline (speedup 1.0000x reference)
"""CrossPixContrastive loss on 8 trn2 NeuronCores.

Math (per batch n, HW=4096, C=256):
  rgb_n = l2norm_C(rgb); ir_n = l2norm_C(ir)
  e[p,q] = exp(20 * <rgb_n[:,p], ir_n[:,q]>)
  S[p] = sum_q e ; M[p] = sum_q e * (rm_p == im_q)
  Ce[q] = sum_p e ; Mc[q] = sum_p e * (rm_p == im_q)
  r_rgb = M/(S+1e-6) ; r_ir = Mc/(Ce+1e-6)
  loss = mean(-log over nonzero of concat(r_rgb, r_ir) * fg)

Sharding: 8 cores = 4 batches x 2 halves of the rgb-pixel axis p.
Host prep: l2-normalize rgb (x20, folding the temperature) and ir, cast
to bf16, build the im broadcast and rm onehot lhsT.
Per-core tiling: 64 tiles of [128p x 1024q] of e; all matmul operands bf16.
  PE  : logit matmul (4x 512-wide, K=128, bf16) + lagged column-sum matmul
        with lhsT = [ones | onehot(rm) x5] -> psum [6,1024] accumulated over p
  ACT : e = Exp(psum logit) -> bf16, fused row-sum accum -> S
  DVE : M row-accum via (im==rm)*e scalar_tensor_tensor, 2048-wide over
        qb pairs (DVE accumulate ops run 1x; wider amortizes overhead)
Host combines the tiny per-core partials into the scalar loss.
"""
import numpy as np
import ml_dtypes

import concourse.bacc as bacc
import concourse.tile as tile
from concourse import mybir
from concourse.bass_utils import run_bass_kernel_spmd

dt = mybir.dt
AF = mybir.ActivationFunctionType
ALU = mybir.AluOpType

N, C, H, W = 4, 256, 64, 64
HW = H * W                      # 4096
PH = HW // 2                    # 2048  p-half per core
NPT = PH // 128                 # 16    p-tiles
QB = 1024                       # q tile width
NQB = HW // QB                  # 4
NCLS = 5
TEMP_INV = 20.0
EPS_DEN = 1e-6
EPS_NORM = 1e-12

_CACHED_NC = None

_TABLES_PATCHED = False


def _patch_activation_tables():
    """Keep Exp only in one table set so the compiler never thrashes
    activation tables."""
    global _TABLES_PATCHED
    if _TABLES_PATCHED:
        return
    _TABLES_PATCHED = True
    import concourse.hw_specs as hw_specs
    import concourse.bacc as _bacc
    orig = hw_specs.get_activation_tables

    def patched(arch):
        tabs = {k: set(v) for k, v in orig(arch).items()}
        exp, ln = AF.Exp, AF.Ln
        for name, fns in tabs.items():
            if name != "natural_log_exp_and_others":
                fns.discard(exp)
                fns.discard(ln)
        return tabs

    hw_specs.get_activation_tables = patched
    if getattr(_bacc, "get_activation_tables", None) is orig:
        _bacc.get_activation_tables = patched


def build_nc():
    _patch_activation_tables()
    nc = bacc.Bacc("TRN2", target_bir_lowering=False, debug=False, num_devices=8)

    rgb_in = nc.dram_tensor("rgb_half", [2, 128, PH], dt.bfloat16, kind="ExternalInput").ap()
    ir_in = nc.dram_tensor("ir_full", [2, 128, HW], dt.bfloat16, kind="ExternalInput").ap()
    im_in = nc.dram_tensor("im_row", [1, HW], dt.bfloat16, kind="ExternalInput").ap()
    rm_in = nc.dram_tensor("rm_cols", [128, NPT], dt.float32, kind="ExternalInput").ap()
    oh_in = nc.dram_tensor("oh_lhsT", [128, NPT * 6], dt.bfloat16, kind="ExternalInput").ap()

    S_out = nc.dram_tensor("S_out", [NPT, 128], dt.float32, kind="ExternalOutput").ap()
    M_out = nc.dram_tensor("M_out", [NPT, 128], dt.float32, kind="ExternalOutput").ap()
    C_out = nc.dram_tensor("C_out", [6, HW], dt.float32, kind="ExternalOutput").ap()

    with tile.TileContext(nc) as tc:
        with tc.tile_pool(name="big", bufs=1) as big, \
             tc.tile_pool(name="epool", bufs=4) as epool, \
             tc.tile_pool(name="dump", bufs=3) as dump, \
             tc.tile_pool(name="psL", bufs=2, space="PSUM") as psL, \
             tc.tile_pool(name="psC", bufs=2, space="PSUM") as psCp:

            # ---------------- loads (order = lead-in criticality) ----------
            rm_c = big.tile([128, NPT], dt.float32, name="rm_c")
            nc.sync.dma_start(rm_c[:], rm_in)
            oh_b = big.tile([128, NPT * 6], dt.bfloat16, name="oh_b")
            nc.sync.dma_start(oh_b[:], oh_in)
            im_row = big.tile([1, HW], dt.bfloat16, name="im_row")
            nc.sync.dma_start(im_row[:], im_in)

            rg0 = big.tile([128, PH], dt.bfloat16, name="rg0")
            rg1 = big.tile([128, PH], dt.bfloat16, name="rg1")
            ir0 = big.tile([128, HW], dt.bfloat16, name="ir0")
            ir1 = big.tile([128, HW], dt.bfloat16, name="ir1")
            # interleave rgb p-chunks with the first ir q-chunk so the
            # first tiles' operands land as early as possible
            for s in range(4):
                for c, rgt in enumerate((rg0, rg1)):
                    nc.sync.dma_start(rgt[:, s * 512:(s + 1) * 512],
                                      rgb_in[c][:, s * 512:(s + 1) * 512])
                if s < 2:
                    for c, irt in enumerate((ir0, ir1)):
                        nc.sync.dma_start(irt[:, s * 512:(s + 1) * 512],
                                          ir_in[c][:, s * 512:(s + 1) * 512])
            for qb in range(1, NQB):
                for c, irt in enumerate((ir0, ir1)):
                    nc.sync.dma_start(irt[:, qb * QB:(qb + 1) * QB],
                                      ir_in[c][:, qb * QB:(qb + 1) * QB])

            # broadcast the ir mask row across partitions per q-chunk (GPS
            # is otherwise idle; saves 1MB of host DMA)
            im_b = big.tile([128, HW], dt.bfloat16, name="im_b")
            for qb in range(NQB):
                nc.gpsimd.partition_broadcast(im_b[:, qb * QB:(qb + 1) * QB],
                                              im_row[:1, qb * QB:(qb + 1) * QB])

            S_stat = big.tile([128, NPT * NQB], dt.float32, name="S_stat")
            M_stat = big.tile([128, NPT * NQB], dt.float32, name="M_stat")
            C_sb = big.tile([6, HW], dt.float32, name="C_sb")

            # ---------------- main loop ----------------
            pending_c = []
            LAG = 2

            def issue_c(entry):
                e_t, pt_, psc_, qb_ = entry
                for h in range(2):
                    nc.tensor.matmul(psc_[:, h * 512:(h + 1) * 512],
                                     oh_b[:, pt_ * 6:(pt_ + 1) * 6],
                                     e_t[:, h * 512:(h + 1) * 512],
                                     start=(pt_ == 0), stop=(pt_ == NPT - 1))
                if pt_ == NPT - 1:
                    nc.vector.tensor_copy(
                        C_sb[:, qb_ * QB:(qb_ + 1) * QB], psc_[:])

            for qb in range(NQB):
                psc = psCp.tile([6, QB], dt.float32, tag="psc")
                for pt in range(NPT):
                    pl = psL.tile([128, QB], dt.float32, tag="pl")
                    for c, (rg, ir) in enumerate(((rg0, ir0), (rg1, ir1))):
                        for h in range(2):
                            q0 = qb * QB + h * 512
                            nc.tensor.matmul(pl[:, h * 512:(h + 1) * 512],
                                             rg[:, pt * 128:(pt + 1) * 128],
                                             ir[:, q0:q0 + 512],
                                             start=(c == 0), stop=(c == 1))
                    if len(pending_c) >= LAG:
                        issue_c(pending_c.pop(0))
                    idx = pt * NQB + qb
                    e_t = epool.tile([128, QB], dt.bfloat16, tag="e")
                    nc.scalar.activation(e_t[:], pl[:], AF.Exp,
                                         accum_out=S_stat[:, idx:idx + 1])
                    num = dump.tile([128, QB], dt.bfloat16, tag="num")
                    nc.vector.scalar_tensor_tensor(
                        out=num[:],
                        in0=im_b[:, qb * QB:(qb + 1) * QB],
                        scalar=rm_c[:, pt:pt + 1],
                        in1=e_t[:],
                        op0=ALU.is_equal, op1=ALU.mult,
                        accum_out=M_stat[:, idx:idx + 1])
                    pending_c.append((e_t, pt, psc, qb))
            while pending_c:
                issue_c(pending_c.pop(0))

            # ---------------- epilogue ----------------
            S_red = big.tile([128, NPT], dt.float32, name="S_red")
            nc.vector.reduce_sum(S_red[:],
                                 S_stat[:].rearrange("p (pt q) -> p pt q", q=NQB),
                                 axis=mybir.AxisListType.X)
            M_red = big.tile([128, NPT], dt.float32, name="M_red")
            nc.vector.reduce_sum(M_red[:],
                                 M_stat[:].rearrange("p (pt q) -> p pt q", q=NQB),
                                 axis=mybir.AxisListType.X)
            nc.sync.dma_start(S_out.rearrange("pt p -> p pt"), S_red[:])
            nc.sync.dma_start(M_out.rearrange("pt p -> p pt"), M_red[:])
            nc.sync.dma_start(C_out, C_sb[:])

    nc.compile()
    return nc


def _get_nc():
    global _CACHED_NC
    if _CACHED_NC is None:
        _CACHED_NC = build_nc()
    return _CACHED_NC


def _build_in_maps(np_inputs):
    bf16 = ml_dtypes.bfloat16
    rgb_map = np.asarray(np_inputs["rgb_map"], dtype=np.float32).reshape(N, C, HW)
    ir_map = np.asarray(np_inputs["ir_map"], dtype=np.float32).reshape(N, C, HW)
    # host-side l2 norm over channels; fold the 1/T=20 into rgb
    rgb_map = (rgb_map * (TEMP_INV / np.maximum(
        np.linalg.norm(rgb_map, axis=1, keepdims=True), EPS_NORM))).astype(bf16)
    ir_map = (ir_map / np.maximum(
        np.linalg.norm(ir_map, axis=1, keepdims=True), EPS_NORM)).astype(bf16)
    rm = np.asarray(np_inputs["rgb_mask"]).reshape(N, HW)
    im = np.asarray(np_inputs["ir_mask"]).reshape(N, HW)
    rm_f = rm.astype(np.float32)
    im_f = im.astype(bf16)

    in_maps = []
    for core in range(8):
        n, h = core // 2, core % 2
        psl = slice(h * PH, (h + 1) * PH)
        rgb_half = np.ascontiguousarray(rgb_map[n, :, psl].reshape(2, 128, PH))
        ir_full = np.ascontiguousarray(ir_map[n].reshape(2, 128, HW))
        im_row = np.ascontiguousarray(im_f[n].reshape(1, HW))
        rm_half = rm_f[n, psl]
        rm_cols = np.ascontiguousarray(rm_half.reshape(NPT, 128).T)
        oh = np.empty((NPT, 128, 6), dtype=np.float32)
        oh[:, :, 0] = 1.0
        rm_tiles = rm_half.reshape(NPT, 128)
        for k in range(NCLS):
            oh[:, :, 1 + k] = (rm_tiles == k)
        oh_lhsT = np.ascontiguousarray(
            oh.transpose(1, 0, 2).reshape(128, NPT * 6)).astype(bf16)
        in_maps.append({
            "rgb_half": rgb_half,
            "ir_full": ir_full,
            "im_row": im_row,
            "rm_cols": rm_cols,
            "oh_lhsT": oh_lhsT,
        })
    return in_maps


def kernel(rgb_map, ir_map, rgb_mask, ir_mask):
    np_inputs = {"rgb_map": rgb_map, "ir_map": ir_map,
                 "rgb_mask": rgb_mask, "ir_mask": ir_mask}
    in_maps = _build_in_maps(np_inputs)
    im = np.asarray(ir_mask).reshape(N, HW)
    rm = np.asarray(rgb_mask).reshape(N, HW)

    nc = _get_nc()
    res = run_bass_kernel_spmd(nc, in_maps, list(range(8)))

    # ---------------- host combine (tiny) ----------------
    entries = []
    for n in range(N):
        rA, rB = res.results[2 * n], res.results[2 * n + 1]
        S = np.concatenate([rA["S_out"].reshape(PH), rB["S_out"].reshape(PH)]).astype(np.float64)
        M = np.concatenate([rA["M_out"].reshape(PH), rB["M_out"].reshape(PH)]).astype(np.float64)
        C6 = rA["C_out"].astype(np.float64) + rB["C_out"].astype(np.float64)
        Ce = C6[0]
        imn = im[n]
        Mc = C6[1 + imn, np.arange(HW)]
        r_rgb = (M / (S + EPS_DEN)) * (rm[n] > 0)
        r_ir = (Mc / (Ce + EPS_DEN)) * (imn > 0)
        entries.append(r_rgb)
        entries.append(r_ir)
    L = np.concatenate(entries)
    nz = L != 0
    total = -np.log(L[nz]).sum() if nz.any() else 0.0
    count = max(float(nz.sum()), 1.0)
    return np.asarray(np.float32(total / count))


if __name__ == "__main__":
    import reference
    inputs = reference.setup_inputs()
    inputs = {k: np.asarray(v) for k, v in inputs.items()}
    out = kernel(**inputs)
    print("kernel:", out)


# revision 22
# speedup vs baseline: 1.0202x; 1.0202x over previous
"""CrossPixContrastive loss on 8 trn2 NeuronCores.

Math (per batch n, HW=4096, C=256):
  rgb_n = l2norm_C(rgb); ir_n = l2norm_C(ir)
  e[p,q] = exp(20 * <rgb_n[:,p], ir_n[:,q]>)
  S[p] = sum_q e ; M[p] = sum_q e * (rm_p == im_q)
  Ce[q] = sum_p e ; Mc[q] = sum_p e * (rm_p == im_q)
  r_rgb = M/(S+1e-6) ; r_ir = Mc/(Ce+1e-6)
  loss = mean(-log over nonzero of concat(r_rgb, r_ir) * fg)

Sharding: 8 cores = 4 batches x 2 halves of the rgb-pixel axis p.
Host prep: l2-normalize rgb (x20, folding the temperature) and ir, cast
to bf16, build the im broadcast and rm onehot lhsT.
Per-core tiling: 64 tiles of [128p x 1024q] of e; all matmul operands bf16.
  PE  : logit matmul (4x 512-wide, K=128, bf16) + lagged column-sum matmul
        with lhsT = [ones | onehot(rm) x5] -> psum [6,1024] accumulated over p
  ACT : e = Exp(psum logit) -> bf16, fused row-sum accum -> S
  DVE : M row-accum via (im==rm)*e scalar_tensor_tensor, 2048-wide over
        qb pairs (DVE accumulate ops run 1x; wider amortizes overhead)
Host combines the tiny per-core partials into the scalar loss.
"""
import numpy as np
import ml_dtypes

import concourse.bacc as bacc
import concourse.tile as tile
from concourse import mybir
from concourse.bass_utils import run_bass_kernel_spmd

dt = mybir.dt
AF = mybir.ActivationFunctionType
ALU = mybir.AluOpType

N, C, H, W = 4, 256, 64, 64
HW = H * W                      # 4096
PH = HW // 2                    # 2048  p-half per core
NPT = PH // 128                 # 16    p-tiles
QB = 1024                       # q tile width
NQB = HW // QB                  # 4
NCLS = 5
TEMP_INV = 20.0
EPS_DEN = 1e-6
EPS_NORM = 1e-12

_CACHED_NC = None

_TABLES_PATCHED = False


def _patch_activation_tables():
    """Keep Exp only in one table set so the compiler never thrashes
    activation tables."""
    global _TABLES_PATCHED
    if _TABLES_PATCHED:
        return
    _TABLES_PATCHED = True
    import concourse.hw_specs as hw_specs
    import concourse.bacc as _bacc
    orig = hw_specs.get_activation_tables

    def patched(arch):
        tabs = {k: set(v) for k, v in orig(arch).items()}
        exp, ln = AF.Exp, AF.Ln
        for name, fns in tabs.items():
            if name != "natural_log_exp_and_others":
                fns.discard(exp)
                fns.discard(ln)
        return tabs

    hw_specs.get_activation_tables = patched
    if getattr(_bacc, "get_activation_tables", None) is orig:
        _bacc.get_activation_tables = patched


_LDW_PATCHED = False


def _patch_ldw_opt():
    """Enable walrus's LDWEIGHTS optimization pass (dedupes/hides weight
    loads); concourse pins it off but ~15% of our PE time is exposed LDW."""
    global _LDW_PATCHED
    if _LDW_PATCHED:
        return
    _LDW_PATCHED = True
    import subprocess
    orig = subprocess.check_call

    def check_call(argv, *a, **kw):
        if isinstance(argv, list) and any("walrus_driver" in str(x) for x in argv[:2]):
            argv = ["--enable-ldw-opt=true" if x == "--enable-ldw-opt=false"
                    else x for x in argv]
        return orig(argv, *a, **kw)

    subprocess.check_call = check_call


def build_nc():
    _patch_activation_tables()
    _patch_ldw_opt()
    nc = bacc.Bacc("TRN2", target_bir_lowering=False, debug=False, num_devices=8)

    rgb_in = nc.dram_tensor("rgb_half", [2, 128, PH], dt.bfloat16, kind="ExternalInput").ap()
    ir_in = nc.dram_tensor("ir_full", [2, 128, HW], dt.bfloat16, kind="ExternalInput").ap()
    im_in = nc.dram_tensor("im_row", [1, HW], dt.bfloat16, kind="ExternalInput").ap()
    rm_in = nc.dram_tensor("rm_cols", [128, NPT], dt.float32, kind="ExternalInput").ap()
    oh_in = nc.dram_tensor("oh_lhsT", [128, NPT * 6], dt.bfloat16, kind="ExternalInput").ap()

    S_out = nc.dram_tensor("S_out", [NPT, 128], dt.float32, kind="ExternalOutput").ap()
    M_out = nc.dram_tensor("M_out", [NPT, 128], dt.float32, kind="ExternalOutput").ap()
    C_out = nc.dram_tensor("C_out", [6, HW], dt.float32, kind="ExternalOutput").ap()

    with tile.TileContext(nc) as tc:
        with tc.tile_pool(name="big", bufs=1) as big, \
             tc.tile_pool(name="epool", bufs=4) as epool, \
             tc.tile_pool(name="dump", bufs=3) as dump, \
             tc.tile_pool(name="psL", bufs=2, space="PSUM") as psL, \
             tc.tile_pool(name="psC", bufs=2, space="PSUM") as psCp:

            # ---------------- loads (order = lead-in criticality) ----------
            rm_c = big.tile([128, NPT], dt.float32, name="rm_c")
            nc.sync.dma_start(rm_c[:], rm_in)
            oh_b = big.tile([128, NPT * 6], dt.bfloat16, name="oh_b")
            nc.sync.dma_start(oh_b[:], oh_in)
            im_row = big.tile([1, HW], dt.bfloat16, name="im_row")
            nc.sync.dma_start(im_row[:], im_in)

            rg0 = big.tile([128, PH], dt.bfloat16, name="rg0")
            rg1 = big.tile([128, PH], dt.bfloat16, name="rg1")
            ir0 = big.tile([128, HW], dt.bfloat16, name="ir0")
            ir1 = big.tile([128, HW], dt.bfloat16, name="ir1")
            # interleave rgb p-chunks with the first ir q-chunk so the
            # first tiles' operands land as early as possible
            for s in range(4):
                for c, rgt in enumerate((rg0, rg1)):
                    nc.sync.dma_start(rgt[:, s * 512:(s + 1) * 512],
                                      rgb_in[c][:, s * 512:(s + 1) * 512])
                if s < 2:
                    for c, irt in enumerate((ir0, ir1)):
                        nc.sync.dma_start(irt[:, s * 512:(s + 1) * 512],
                                          ir_in[c][:, s * 512:(s + 1) * 512])
            for qb in range(1, NQB):
                for c, irt in enumerate((ir0, ir1)):
                    nc.sync.dma_start(irt[:, qb * QB:(qb + 1) * QB],
                                      ir_in[c][:, qb * QB:(qb + 1) * QB])

            # broadcast the ir mask row across partitions per q-chunk (GPS
            # is otherwise idle; saves 1MB of host DMA)
            im_b = big.tile([128, HW], dt.bfloat16, name="im_b")
            for qb in range(NQB):
                nc.gpsimd.partition_broadcast(im_b[:, qb * QB:(qb + 1) * QB],
                                              im_row[:1, qb * QB:(qb + 1) * QB])

            S_stat = big.tile([128, NPT * NQB], dt.float32, name="S_stat")
            M_stat = big.tile([128, NPT * NQB], dt.float32, name="M_stat")
            C_sb = big.tile([6, HW], dt.float32, name="C_sb")

            # ---------------- main loop ----------------
            pending_c = []
            LAG = 2

            def issue_c(entry):
                e_t, pt_, psc_, qb_ = entry
                for h in range(2):
                    nc.tensor.matmul(psc_[:, h * 512:(h + 1) * 512],
                                     oh_b[:, pt_ * 6:(pt_ + 1) * 6],
                                     e_t[:, h * 512:(h + 1) * 512],
                                     start=(pt_ == 0), stop=(pt_ == NPT - 1))
                if pt_ == NPT - 1:
                    nc.vector.tensor_copy(
                        C_sb[:, qb_ * QB:(qb_ + 1) * QB], psc_[:])

            for qb in range(NQB):
                psc = psCp.tile([6, QB], dt.float32, tag="psc")
                for pt in range(NPT):
                    pl = psL.tile([128, QB], dt.float32, tag="pl")
                    for c, (rg, ir) in enumerate(((rg0, ir0), (rg1, ir1))):
                        for h in range(2):
                            q0 = qb * QB + h * 512
                            nc.tensor.matmul(pl[:, h * 512:(h + 1) * 512],
                                             rg[:, pt * 128:(pt + 1) * 128],
                                             ir[:, q0:q0 + 512],
                                             start=(c == 0), stop=(c == 1))
                    if len(pending_c) >= LAG:
                        issue_c(pending_c.pop(0))
                    idx = pt * NQB + qb
                    e_t = epool.tile([128, QB], dt.bfloat16, tag="e")
                    nc.scalar.activation(e_t[:], pl[:], AF.Exp,
                                         accum_out=S_stat[:, idx:idx + 1])
                    num = dump.tile([128, QB], dt.bfloat16, tag="num")
                    nc.vector.scalar_tensor_tensor(
                        out=num[:],
                        in0=im_b[:, qb * QB:(qb + 1) * QB],
                        scalar=rm_c[:, pt:pt + 1],
                        in1=e_t[:],
                        op0=ALU.is_equal, op1=ALU.mult,
                        accum_out=M_stat[:, idx:idx + 1])
                    pending_c.append((e_t, pt, psc, qb))
            while pending_c:
                issue_c(pending_c.pop(0))

            # ---------------- epilogue ----------------
            S_red = big.tile([128, NPT], dt.float32, name="S_red")
            nc.vector.reduce_sum(S_red[:],
                                 S_stat[:].rearrange("p (pt q) -> p pt q", q=NQB),
                                 axis=mybir.AxisListType.X)
            M_red = big.tile([128, NPT], dt.float32, name="M_red")
            nc.vector.reduce_sum(M_red[:],
                                 M_stat[:].rearrange("p (pt q) -> p pt q", q=NQB),
                                 axis=mybir.AxisListType.X)
            nc.sync.dma_start(S_out.rearrange("pt p -> p pt"), S_red[:])
            nc.sync.dma_start(M_out.rearrange("pt p -> p pt"), M_red[:])
            nc.sync.dma_start(C_out, C_sb[:])

    nc.compile()
    return nc


def _get_nc():
    global _CACHED_NC
    if _CACHED_NC is None:
        _CACHED_NC = build_nc()
    return _CACHED_NC


def _build_in_maps(np_inputs):
    bf16 = ml_dtypes.bfloat16
    rgb_map = np.asarray(np_inputs["rgb_map"], dtype=np.float32).reshape(N, C, HW)
    ir_map = np.asarray(np_inputs["ir_map"], dtype=np.float32).reshape(N, C, HW)
    # host-side l2 norm over channels; fold the 1/T=20 into rgb
    rgb_map = (rgb_map * (TEMP_INV / np.maximum(
        np.linalg.norm(rgb_map, axis=1, keepdims=True), EPS_NORM))).astype(bf16)
    ir_map = (ir_map / np.maximum(
        np.linalg.norm(ir_map, axis=1, keepdims=True), EPS_NORM)).astype(bf16)
    rm = np.asarray(np_inputs["rgb_mask"]).reshape(N, HW)
    im = np.asarray(np_inputs["ir_mask"]).reshape(N, HW)
    rm_f = rm.astype(np.float32)
    im_f = im.astype(bf16)

    in_maps = []
    for core in range(8):
        n, h = core // 2, core % 2
        psl = slice(h * PH, (h + 1) * PH)
        rgb_half = np.ascontiguousarray(rgb_map[n, :, psl].reshape(2, 128, PH))
        ir_full = np.ascontiguousarray(ir_map[n].reshape(2, 128, HW))
        im_row = np.ascontiguousarray(im_f[n].reshape(1, HW))
        rm_half = rm_f[n, psl]
        rm_cols = np.ascontiguousarray(rm_half.reshape(NPT, 128).T)
        oh = np.empty((NPT, 128, 6), dtype=np.float32)
        oh[:, :, 0] = 1.0
        rm_tiles = rm_half.reshape(NPT, 128)
        for k in range(NCLS):
            oh[:, :, 1 + k] = (rm_tiles == k)
        oh_lhsT = np.ascontiguousarray(
            oh.transpose(1, 0, 2).reshape(128, NPT * 6)).astype(bf16)
        in_maps.append({
            "rgb_half": rgb_half,
            "ir_full": ir_full,
            "im_row": im_row,
            "rm_cols": rm_cols,
            "oh_lhsT": oh_lhsT,
        })
    return in_maps


def kernel(rgb_map, ir_map, rgb_mask, ir_mask):
    np_inputs = {"rgb_map": rgb_map, "ir_map": ir_map,
                 "rgb_mask": rgb_mask, "ir_mask": ir_mask}
    in_maps = _build_in_maps(np_inputs)
    im = np.asarray(ir_mask).reshape(N, HW)
    rm = np.asarray(rgb_mask).reshape(N, HW)

    nc = _get_nc()
    res = run_bass_kernel_spmd(nc, in_maps, list(range(8)))

    # ---------------- host combine (tiny) ----------------
    entries = []
    for n in range(N):
        rA, rB = res.results[2 * n], res.results[2 * n + 1]
        S = np.concatenate([rA["S_out"].reshape(PH), rB["S_out"].reshape(PH)]).astype(np.float64)
        M = np.concatenate([rA["M_out"].reshape(PH), rB["M_out"].reshape(PH)]).astype(np.float64)
        C6 = rA["C_out"].astype(np.float64) + rB["C_out"].astype(np.float64)
        Ce = C6[0]
        imn = im[n]
        Mc = C6[1 + imn, np.arange(HW)]
        r_rgb = (M / (S + EPS_DEN)) * (rm[n] > 0)
        r_ir = (Mc / (Ce + EPS_DEN)) * (imn > 0)
        entries.append(r_rgb)
        entries.append(r_ir)
    L = np.concatenate(entries)
    nz = L != 0
    total = -np.log(L[nz]).sum() if nz.any() else 0.0
    count = max(float(nz.sum()), 1.0)
    return np.asarray(np.float32(total / count))


if __name__ == "__main__":
    import reference
    inputs = reference.setup_inputs()
    inputs = {k: np.asarray(v) for k, v in inputs.items()}
    out = kernel(**inputs)
    print("kernel:", out)
